# revision 10
# baseline (speedup 1.0000x reference)
"""Trainium2 Bass kernel for nn_Attention_49134425866421.

Dense transformer attention block:
  qkv = x @ W_qkv + b_qkv  -> partial RoPE on q,k -> softmax attention -> out proj.

Shapes (hardcoded): B=4, N=2048, C=768, H=12, D=64, fp32.

Sharding: 8 cores = (batch b in 0..3) x (head-group g in 0..1, 6 heads each).
Each core computes q/k/v projections for its 6 heads, attention, and a partial
output projection (row-parallel over head dims). Host sums the two partials
per batch and adds the effective bias.

Design notes (v4):
- A@V runs "swapped": pt (exp scores [k,q]) is the stationary operand, V the
  moving one -> out [q-tile 128, 65] costs only 65 PE rows/matmul (4x fewer
  PE cycles than the [65, 512] orientation). Column 64 of V is ones so the
  softmax denominator accumulates in out column 64.
- Softmax exp is split across three engines: ACT does exact Exp; DVE and
  GPSIMD compute a Schraudolph-style exp: bits=rne(23.0831*s+16250) as int16,
  bit-cast to bf16 (max rel err ~3%, harmless inside softmax).
- The denominator lands per-PARTITION (out [q,65]), so rescale is a cheap
  reciprocal + per-partition tensor_scalar multiply.
- attn output [q, c] is transposed back to [c, q] for the row-parallel out
  projection via DMA XBAR transpose (idle DMA engines, 2-byte dtype).
- v-projection bias is folded into the host-side output bias: since softmax
  rows sum to 1, out = A@(v+b) = A@v + b, and b @ W_proj + b_proj is constant.
- PSUM budget: 4 score banks + 2 acc banks + 2 proj banks = 8. A@V for the
  4 q-subtiles of a 512-chunk runs in two phases of 2 accumulators; pt tiles
  are retained in SBUF for the second phase.

RoPE trick: rotate_half is a cross-partition half-swap; done via SBUF->SBUF
DMA of (q * m2s) where m2s = pre-swapped signed sin table, so
q_rope = q*cos + swap(q*m2s). Special (non-rotated) tokens handled by padding
cos=1,sin=0 rows host-side. Softmax without max-subtraction (scores are
N(0,~1); exp never overflows); scale 1/8 folded into the exp.
"""

import os
import sys

import numpy as np

try:
    import concourse.bass as bass  # noqa: F401
except ImportError:
    sys.path.insert(0, "/opt/trn_rl_repo")

import ml_dtypes

B, N, C, H, D = 4, 2048, 768, 12, 64
HPC = 6          # heads per core
NPAIR = 3        # head pairs per core
P = 128
NT = N // P      # 16 token tiles
TC = 512         # token chunk for matmul free dim
NTC = N // TC    # 4

# Schraudolph bf16 exp: bits = rne(A*s_raw + Bc) -> int16 -> bf16
SCH_A = 23.08312065          # 128 * log2(e) * 0.125
SCH_B = 16250.0              # 16256 + corr(-6)

# exp engine assignment per jt (A=ACT exact, D=DVE schraudolph).
# GPSIMD cannot read PSUM, so it gets rope work instead of exp tiles.
ENG16 = "AADADAADADAADAAD"

_NC_CACHE = {}
LAST_RESULTS = None  # BassKernelResults stash for test.py


def _build_nc():
    from contextlib import ExitStack

    import concourse.bass as bass
    import concourse.bacc as bacc
    import concourse.mybir as mybir
    import concourse.tile as tile

    f32 = mybir.dt.float32
    f32r = mybir.dt.float32r
    bf16 = mybir.dt.bfloat16
    i16 = mybir.dt.int16
    EXP = mybir.ActivationFunctionType.Exp
    MULT = mybir.AluOpType.mult
    ADD = mybir.AluOpType.add

    nc = bacc.Bacc(None, target_bir_lowering=False)

    xT_d = nc.dram_tensor("xT", [C, N], f32r, kind="ExternalInput")
    wqk_d = nc.dram_tensor("w_qk", [P, 6, 768], f32r, kind="ExternalInput")
    wv_d = nc.dram_tensor("w_v", [P, 6, 384], f32r, kind="ExternalInput")
    wp_d = nc.dram_tensor("w_p", [P, 3, 768], bf16, kind="ExternalInput")
    bqkt_d = nc.dram_tensor("b_qk_t", [P, 6], f32, kind="ExternalInput")
    cos_d = nc.dram_tensor("cos_tab", [P, N], f32, kind="ExternalInput")
    m2s_d = nc.dram_tensor("m2s_tab", [P, N], f32, kind="ExternalInput")
    y_d = nc.dram_tensor("y", [N, C], f32, kind="ExternalOutput")

    with tile.TileContext(nc) as tc, ExitStack() as ctx:
        singles = ctx.enter_context(tc.tile_pool(name="singles", bufs=1))
        mm_ps = ctx.enter_context(tc.tile_pool(name="mm_ps", bufs=2, space="PSUM"))
        st_ps = ctx.enter_context(tc.tile_pool(name="st_ps", bufs=4, space="PSUM"))
        acc_ps = ctx.enter_context(tc.tile_pool(name="acc_ps", bufs=2, space="PSUM"))
        rope_tmp = ctx.enter_context(tc.tile_pool(name="rope_tmp", bufs=2))
        pt_pool = ctx.enter_context(tc.tile_pool(name="pt", bufs=18))
        aq_pool = ctx.enter_context(tc.tile_pool(name="aq", bufs=6))
        rec_pool = ctx.enter_context(tc.tile_pool(name="rec", bufs=4))
        y_pool = ctx.enter_context(tc.tile_pool(name="yout", bufs=2))

        # ---- static SBUF tensors ----
        xT = singles.tile([P, 6, N], f32r)
        wqk = singles.tile([P, 6, 768], f32r)
        wv = singles.tile([P, 6, 384], f32r)
        wp = singles.tile([P, 3, 768], bf16)
        cosT = singles.tile([P, N], f32)
        m2sT = singles.tile([P, N], f32)
        bqkt = singles.tile([P, 6], f32)
        qT = singles.tile([P, NPAIR, N], f32r)
        kT = singles.tile([P, NPAIR, N], f32r)
        Vt = singles.tile([P, NT, HPC, D + 1], bf16)
        attnT = singles.tile([P, NPAIR, N], bf16)

        xT_r = xT_d.rearrange("(ko p) t -> p ko t", p=P)
        for ko in range(6):
            nc.sync.dma_start(xT[:, ko, :], xT_r[:, ko, :])
        nc.scalar.dma_start(wqk[:], wqk_d[:])
        nc.sync.dma_start(wv[:], wv_d[:])
        nc.sync.dma_start(bqkt[:], bqkt_d[:])
        nc.scalar.dma_start(cosT[:], cos_d[:])
        nc.scalar.dma_start(m2sT[:], m2s_d[:])
        nc.gpsimd.memset(Vt[:], 1.0)

        def emit_qk(hp):
            for tcu in range(NTC):
                tsl = slice(tcu * TC, (tcu + 1) * TC)
                for mt in (3 + hp, hp):  # k pair first, then q pair
                    dst = qT if mt < 3 else kT
                    ps = mm_ps.tile([P, TC], f32, tag="mm")
                    for ko in range(6):
                        nc.tensor.matmul(
                            ps,
                            lhsT=wqk[:, ko, mt * P : (mt + 1) * P],
                            rhs=xT[:, ko, tsl],
                            start=(ko == 0),
                            stop=(ko == 5),
                        )
                    # bias add on DVE, then rope: dst = pb*cos + swap(pb*m2s)
                    pb = rope_tmp.tile([P, TC], f32, tag="pb")
                    qs = rope_tmp.tile([P, TC], f32, tag="qs")
                    qsw = rope_tmp.tile([P, TC], f32, tag="qsw")
                    nc.vector.tensor_scalar_add(
                        out=pb[:], in0=ps[:], scalar1=bqkt[:, mt : mt + 1]
                    )
                    nc.gpsimd.tensor_mul(out=qs[:], in0=pb[:], in1=m2sT[:, tsl])
                    nc.vector.tensor_mul(
                        out=dst[:, hp, tsl], in0=pb[:], in1=cosT[:, tsl]
                    )
                    for blk in range(4):
                        sp = [1, 0, 3, 2][blk] * 32
                        nc.sync.dma_start(
                            out=qsw[blk * 32 : blk * 32 + 32, :],
                            in_=qs[sp : sp + 32, :],
                        )
                    nc.gpsimd.tensor_add(
                        out=dst[:, hp, tsl], in0=dst[:, hp, tsl], in1=qsw[:]
                    )

        emit_qk(0)
        nc.sync.dma_start(wp[:], wp_d[:])

        # ---- V projection (natural layout [t, h, d]), per token-tile;
        # interleaved into the first attention pass. Bias folded host-side ----
        def emit_v(tt):
            ps = mm_ps.tile([P, TC], f32, tag="mm")
            vps = ps[:, :384]
            for ko in range(6):
                nc.tensor.matmul(
                    vps,
                    lhsT=xT[:, ko, tt * P : (tt + 1) * P],
                    rhs=wv[:, ko, :],
                    start=(ko == 0),
                    stop=(ko == 5),
                )
            nc.scalar.copy(
                out=Vt[:, tt, :, :D],
                in_=vps.rearrange("p (h d) -> p h d", h=HPC),
            )

        # ---- attention ----
        for hp in range(NPAIR):
            if hp > 0:
                emit_qk(hp)
            for ic in range(NTC):
                isl = slice(ic * TC, (ic + 1) * TC)
                # attn output tiles [q 128, two heads d 128] for this chunk
                aqs = [aq_pool.tile([P, P], bf16, tag="aq", name=f"aq{i}")
                       for i in range(4)]
                for hh in range(2):
                    head = 2 * hp + hh
                    hsl = slice(hh * D, (hh + 1) * D)
                    pts = []
                    acc = [None, None]
                    for jt in range(NT):
                        if hp == 0 and hh == 0 and ic == 0:
                            emit_v(jt)
                        st = st_ps.tile([P, TC], f32, tag="st")
                        nc.tensor.matmul(
                            st,
                            lhsT=kT[hsl, hp, jt * P : (jt + 1) * P],
                            rhs=qT[hsl, hp, isl],
                            start=True,
                            stop=True,
                            tile_position=(hh * D, 0),
                        )
                        pt = pt_pool.tile([P, TC], bf16, tag="pt")
                        eng = ENG16[jt]
                        if eng == "A":
                            nc.scalar.activation(pt[:], st[:], EXP, scale=0.125)
                        else:
                            nc.vector.tensor_scalar(
                                out=pt[:].bitcast(i16), in0=st[:],
                                scalar1=SCH_A, scalar2=SCH_B,
                                op0=MULT, op1=ADD,
                            )
                        pts.append(pt)
                        # phase A: first two q-subtiles accumulate immediately
                        if jt == 0:
                            acc[0] = acc_ps.tile([P, TC], f32, tag="acc", name="accA")
                            acc[1] = acc_ps.tile([P, TC], f32, tag="acc", name="accB")
                        for qs in range(2):
                            nc.tensor.matmul(
                                acc[qs][:, : D + 1],
                                lhsT=pt[:, qs * P : (qs + 1) * P],
                                rhs=Vt[:, jt, head, :],
                                start=(jt == 0),
                                stop=(jt == NT - 1),
                            )

                    def rescale(qs, acc_t):
                        rec = rec_pool.tile([P, 1], f32, tag="rec")
                        nc.vector.reciprocal(out=rec[:], in_=acc_t[:, D : D + 1])
                        nc.vector.tensor_scalar_mul(
                            out=aqs[qs][:, hsl], in0=acc_t[:, :D], scalar1=rec[:]
                        )

                    for qs in range(2):
                        rescale(qs, acc[qs])
                    # phase B: remaining two q-subtiles from retained pt tiles
                    acc[0] = acc_ps.tile([P, TC], f32, tag="acc", name="accC")
                    acc[1] = acc_ps.tile([P, TC], f32, tag="acc", name="accD")
                    for jt in range(NT):
                        for qs in (2, 3):
                            nc.tensor.matmul(
                                acc[qs - 2][:, : D + 1],
                                lhsT=pts[jt][:, qs * P : (qs + 1) * P],
                                rhs=Vt[:, jt, head, :],
                                start=(jt == 0),
                                stop=(jt == NT - 1),
                            )
                    for qs in (2, 3):
                        rescale(qs, acc[qs - 2])

                # both heads of the pair done for this chunk: transpose
                # [q, c-pair] -> attnT [c-pair, q] via DMA XBAR
                for qs in range(4):
                    tt = ic * 4 + qs
                    nc.sync.dma_start(
                        out=attnT[:, hp, tt * P : (tt + 1) * P],
                        in_=aqs[qs][:],
                        transpose=True,
                    )

        # ---- output projection (row-parallel partial) ----
        for tt in range(NT):
            for ch in range(2):
                ps = mm_ps.tile([P, TC], f32, tag="mm")
                yps = ps[:, :384]
                for ko in range(3):
                    nc.tensor.matmul(
                        yps,
                        lhsT=attnT[:, ko, tt * P : (tt + 1) * P],
                        rhs=wp[:, ko, ch * 384 : (ch + 1) * 384],
                        start=(ko == 0),
                        stop=(ko == 2),
                    )
                yt = y_pool.tile([P, 384], f32, tag="yt")
                nc.scalar.copy(out=yt[:], in_=yps)
                nc.sync.dma_start(
                    out=y_d[tt * P : (tt + 1) * P, ch * 384 : (ch + 1) * 384],
                    in_=yt[:],
                )

    nc.finalize()
    return nc


def _host_inputs(x, rope_cos, rope_sin, W_qkv, b_qkv, W_proj, b_proj, num_special):
    ns = int(num_special)
    cos_pad = np.ones((N, D), np.float32)
    sin_pad = np.zeros((N, D), np.float32)
    cos_pad[ns:] = rope_cos
    sin_pad[ns:] = rope_sin
    # m2s[t, d] = +sin[t, d+32] (d<32) else -sin[t, d-32]
    m2s = np.empty_like(sin_pad)
    m2s[:, : D // 2] = sin_pad[:, D // 2 :]
    m2s[:, D // 2 :] = -sin_pad[:, : D // 2]
    cos_tab = np.tile(np.ascontiguousarray(cos_pad.T), (2, 1))
    m2s_tab = np.tile(np.ascontiguousarray(m2s.T), (2, 1))

    in_maps = []
    for core in range(8):
        b, g = core // 2, core % 2
        hs = list(range(HPC * g, HPC * g + HPC))
        cols_qk = []
        for mt in range(6):
            s, hp = (0, mt) if mt < 3 else (1, mt - 3)
            for half in range(2):
                h = hs[2 * hp + half]
                cols_qk.extend(s * 768 + h * 64 + d for d in range(D))
        cols_qk = np.array(cols_qk)
        cols_v = np.array([2 * 768 + hs[i // 64] * 64 + (i % 64) for i in range(384)])
        rows_p = np.array(
            [hs[2 * ko + half] * 64 + d
             for ko in range(3) for half in range(2) for d in range(D)]
        )
        in_maps.append({
            "xT": np.ascontiguousarray(x[b].T),
            "w_qk": np.ascontiguousarray(
                W_qkv[:, cols_qk].reshape(6, P, 768).transpose(1, 0, 2)),
            "w_v": np.ascontiguousarray(
                W_qkv[:, cols_v].reshape(6, P, 384).transpose(1, 0, 2)),
            "w_p": np.ascontiguousarray(
                W_proj[rows_p].reshape(3, P, 768).transpose(1, 0, 2)
            ).astype(ml_dtypes.bfloat16),
            "b_qk_t": np.ascontiguousarray(
                b_qkv[cols_qk].reshape(6, P).T),
            "cos_tab": cos_tab,
            "m2s_tab": m2s_tab,
        })
    return in_maps


def kernel(x, rope_cos, rope_sin, W_qkv, b_qkv, W_proj, b_proj, num_special):
    global LAST_RESULTS
    from concourse.bass_utils import run_bass_kernel_spmd

    x = np.asarray(x, np.float32)
    W_qkv = np.asarray(W_qkv, np.float32)
    b_qkv = np.asarray(b_qkv, np.float32)
    W_proj = np.asarray(W_proj, np.float32)
    b_proj = np.asarray(b_proj, np.float32)
    if "nc" not in _NC_CACHE:
        _NC_CACHE["nc"] = _build_nc()
    nc = _NC_CACHE["nc"]

    in_maps = _host_inputs(
        x, np.asarray(rope_cos, np.float32), np.asarray(rope_sin, np.float32),
        W_qkv, b_qkv, W_proj, b_proj, num_special,
    )
    trace = bool(int(os.environ.get("KERNEL_TRACE", "0")))
    res = run_bass_kernel_spmd(nc, in_maps, core_ids=list(range(8)), trace=trace)
    LAST_RESULTS = res

    # v-bias folded through the projection: softmax rows sum to 1, so
    # A@(v+b_v) = A@v + b_v; (b_v @ W_proj + b_proj) is a constant row.
    bp_eff = (b_qkv[2 * C :].astype(np.float64) @ W_proj.astype(np.float64)
              + b_proj.astype(np.float64)).astype(np.float32)
    out = np.empty((B, N, C), np.float32)
    for b in range(B):
        out[b] = res.results[2 * b]["y"] + res.results[2 * b + 1]["y"] + bp_eff
    return out


# revision 12
# speedup vs baseline: 1.1560x; 1.1560x over previous
"""Trainium2 Bass kernel for nn_Attention_49134425866421.

Dense transformer attention block:
  qkv = x @ W_qkv + b_qkv  -> partial RoPE on q,k -> softmax attention -> out proj.

Shapes (hardcoded): B=4, N=2048, C=768, H=12, D=64, fp32.

Sharding: 8 cores = (batch b in 0..3) x (head-group g in 0..1, 6 heads each).
Each core computes q/k/v projections for its 6 heads, attention, and a partial
output projection (row-parallel over head dims). Host sums the two partials
per batch and adds the effective bias.

Design notes (v5):
- A@V runs "swapped": pt (exp scores [k,q]) is the stationary operand, V the
  moving one -> out [q-tile 128, 65] costs only 65 PE rows/matmul (4x fewer
  PE cycles than the [65, 512] orientation). Column 64 of V is ones so the
  softmax denominator accumulates in out column 64.
- Softmax exp is split across ACT (exact Exp) and DVE (Schraudolph exp:
  bits=rne(23.0831*s+16250) as int16, bitcast bf16; ~3% max rel err,
  harmless inside softmax). GPSIMD cannot touch PSUM, so it instead runs
  the SBUF-only rope multiplies/adds.
- rotate_half runs on the PE: a signed permutation matrix S (stationary)
  against the pre-rope projection; kills the 96 SBUF->SBUF swap DMAs that
  saturated the HWDGE in v4.
- The A@V denominator lands per-PARTITION, so rescale is reciprocal +
  per-partition tensor_scalar multiply on DVE.
- attn output [q, c] transposes back to [c, q] via DMA XBAR (idle DMA HW).
- v-projection bias folds into the host-side output bias (softmax rows sum
  to 1), and out-proj for the last head-pair interleaves with its attention.
- PSUM: 2 proj banks + 3 score banks + 3 acc banks = 8. A@V phase A covers
  q-subtiles 0-2; phase B covers subtile 3 from SBUF-retained pt tiles.
"""

import os
import sys

import numpy as np

try:
    import concourse.bass as bass  # noqa: F401
except ImportError:
    sys.path.insert(0, "/opt/trn_rl_repo")

import ml_dtypes

B, N, C, H, D = 4, 2048, 768, 12, 64
HPC = 6          # heads per core
NPAIR = 3        # head pairs per core
P = 128
NT = N // P      # 16 token tiles
TC = 512         # token chunk for matmul free dim
NTC = N // TC    # 4

# Schraudolph bf16 exp: bits = rne(A*s_raw + Bc) -> int16 -> bf16
SCH_A = 23.08312065          # 128 * log2(e) * 0.125
SCH_B = 16250.0              # 16256 + corr(-6)

# exp engine assignment per jt (A=ACT exact, D=DVE schraudolph)
ENG16 = "AADADAADADAADAAD"

_NC_CACHE = {}
LAST_RESULTS = None  # BassKernelResults stash for test.py


def _build_nc():
    from contextlib import ExitStack

    import concourse.bass as bass
    import concourse.bacc as bacc
    import concourse.mybir as mybir
    import concourse.tile as tile

    f32 = mybir.dt.float32
    f32r = mybir.dt.float32r
    bf16 = mybir.dt.bfloat16
    i16 = mybir.dt.int16
    EXP = mybir.ActivationFunctionType.Exp
    MULT = mybir.AluOpType.mult
    ADD = mybir.AluOpType.add

    nc = bacc.Bacc(None, target_bir_lowering=False)

    xT_d = nc.dram_tensor("xT", [C, N], f32r, kind="ExternalInput")
    wqk_d = nc.dram_tensor("w_qk", [P, 6, 768], f32r, kind="ExternalInput")
    wv_d = nc.dram_tensor("w_v", [P, 6, 384], f32r, kind="ExternalInput")
    wp_d = nc.dram_tensor("w_p", [P, 3, 768], bf16, kind="ExternalInput")
    bqkt_d = nc.dram_tensor("b_qk_t", [P, 6], f32, kind="ExternalInput")
    cos_d = nc.dram_tensor("cos_tab", [P, N], f32, kind="ExternalInput")
    sin_d = nc.dram_tensor("sin_tab", [P, N], f32, kind="ExternalInput")
    sg_d = nc.dram_tensor("swap_sign", [P, P], f32r, kind="ExternalInput")
    y_d = nc.dram_tensor("y", [N, C], f32, kind="ExternalOutput")

    with tile.TileContext(nc) as tc, ExitStack() as ctx:
        singles = ctx.enter_context(tc.tile_pool(name="singles", bufs=1))
        mm_ps = ctx.enter_context(tc.tile_pool(name="mm_ps", bufs=2, space="PSUM"))
        st_ps = ctx.enter_context(tc.tile_pool(name="st_ps", bufs=3, space="PSUM"))
        acc_ps = ctx.enter_context(tc.tile_pool(name="acc_ps", bufs=3, space="PSUM"))
        rope_tmp = ctx.enter_context(tc.tile_pool(name="rope_tmp", bufs=2))
        pt_pool = ctx.enter_context(tc.tile_pool(name="pt", bufs=18))
        aq_pool = ctx.enter_context(tc.tile_pool(name="aq", bufs=6))
        rec_pool = ctx.enter_context(tc.tile_pool(name="rec", bufs=4))
        y_pool = ctx.enter_context(tc.tile_pool(name="yout", bufs=2))

        # ---- static SBUF tensors ----
        xT = singles.tile([P, 6, N], f32r)
        wqk = singles.tile([P, 6, 768], f32r)
        wv = singles.tile([P, 6, 384], f32r)
        wp = singles.tile([P, 3, 768], bf16)
        cosT = singles.tile([P, N], f32)
        sinT = singles.tile([P, N], f32)
        bqkt = singles.tile([P, 6], f32)
        sg = singles.tile([P, P], f32r)
        qT = singles.tile([P, NPAIR, N], f32r)
        kT = singles.tile([P, NPAIR, N], f32r)
        Vt = singles.tile([P, NT, HPC, D + 1], bf16)
        attnT = singles.tile([P, NPAIR, N], bf16)

        # xT arrives token-chunk-major so the first projection slot can start
        # after ~1/4 of the load
        xT_r = xT_d.rearrange("(ko p) t -> p ko t", p=P)
        for ko in range(6):
            nc.sync.dma_start(xT[:, ko, :TC], xT_r[:, ko, :TC])
        nc.sync.dma_start(sg[:], sg_d[:])
        for ko in range(6):
            nc.sync.dma_start(xT[:, ko, TC:], xT_r[:, ko, TC:])
        nc.scalar.dma_start(wqk[:], wqk_d[:])
        nc.scalar.dma_start(cosT[:], cos_d[:])
        nc.scalar.dma_start(sinT[:], sin_d[:])
        nc.scalar.dma_start(bqkt[:], bqkt_d[:])
        nc.scalar.dma_start(wv[:], wv_d[:])
        nc.gpsimd.memset(Vt[:], 1.0)

        # one projection slot: q or k pair for head-pair hp, token chunk tcu.
        # rope via PE swap: dst = (ps+b)*cos + (S@(ps+b))*sin
        def emit_slot(hp, tcu, which):
            tsl = slice(tcu * TC, (tcu + 1) * TC)
            mt = hp if which == "q" else 3 + hp
            dst = qT if which == "q" else kT
            ps = mm_ps.tile([P, TC], f32, tag="mm", name="ps_proj")
            for ko in range(6):
                nc.tensor.matmul(
                    ps,
                    lhsT=wqk[:, ko, mt * P : (mt + 1) * P],
                    rhs=xT[:, ko, tsl],
                    start=(ko == 0),
                    stop=(ko == 5),
                )
            pb = rope_tmp.tile([P, TC], f32r, tag="pb")
            t1 = rope_tmp.tile([P, TC], f32, tag="t1")
            t2 = rope_tmp.tile([P, TC], f32, tag="t2")
            nc.vector.tensor_scalar_add(
                out=pb[:], in0=ps[:], scalar1=bqkt[:, mt : mt + 1]
            )
            sw = mm_ps.tile([P, TC], f32, tag="mm", name="ps_swap")
            nc.tensor.matmul(sw, lhsT=sg[:], rhs=pb[:], start=True, stop=True)
            nc.gpsimd.tensor_mul(out=t1[:], in0=pb[:], in1=cosT[:, tsl])
            nc.vector.tensor_mul(out=t2[:], in0=sw[:], in1=sinT[:, tsl])
            nc.gpsimd.tensor_add(out=dst[:, hp, tsl], in0=t1[:], in1=t2[:])

        def emit_qk(hp):
            for tcu in range(NTC):
                emit_slot(hp, tcu, "k")
                emit_slot(hp, tcu, "q")

        emit_qk(0)
        nc.sync.dma_start(wp[:], wp_d[:])

        # ---- V projection (natural layout [t, h, d]), per token-tile;
        # interleaved into the first attention pass. Bias folded host-side ----
        def emit_v(tt):
            ps = mm_ps.tile([P, TC], f32, tag="mm", name="ps_v")
            vps = ps[:, :384]
            for ko in range(6):
                nc.tensor.matmul(
                    vps,
                    lhsT=xT[:, ko, tt * P : (tt + 1) * P],
                    rhs=wv[:, ko, :],
                    start=(ko == 0),
                    stop=(ko == 5),
                )
            nc.scalar.copy(
                out=Vt[:, tt, :, :D],
                in_=vps.rearrange("p (h d) -> p h d", h=HPC),
            )

        # ---- output projection for one token tile (both halves, 1 DMA) ----
        def emit_proj(tt):
            yt = y_pool.tile([P, 768], f32, tag="yt")
            for ch in range(2):
                ps = mm_ps.tile([P, TC], f32, tag="mm", name="ps_o")
                yps = ps[:, :384]
                for ko in range(3):
                    nc.tensor.matmul(
                        yps,
                        lhsT=attnT[:, ko, tt * P : (tt + 1) * P],
                        rhs=wp[:, ko, ch * 384 : (ch + 1) * 384],
                        start=(ko == 0),
                        stop=(ko == 2),
                    )
                if ch == 0:
                    nc.vector.tensor_copy(out=yt[:, :384], in_=yps)
                else:
                    nc.scalar.copy(out=yt[:, 384:], in_=yps)
            nc.sync.dma_start(out=y_d[tt * P : (tt + 1) * P, :], in_=yt[:])

        # ---- attention ----
        # (projection slots for hp+1 are interleaved into hp's attention)
        for hp in range(NPAIR):
            for ic in range(NTC):
                isl = slice(ic * TC, (ic + 1) * TC)
                aqs = [aq_pool.tile([P, P], bf16, tag="aq", name=f"aq{i}")
                       for i in range(4)]
                for hh in range(2):
                    head = 2 * hp + hh
                    hsl = slice(hh * D, (hh + 1) * D)
                    pts = []
                    acc = [None] * 4
                    for jt in range(NT):
                        if hp == 0 and hh == 0 and ic == 0:
                            emit_v(jt)
                        st = st_ps.tile([P, TC], f32, tag="st")
                        nc.tensor.matmul(
                            st,
                            lhsT=kT[hsl, hp, jt * P : (jt + 1) * P],
                            rhs=qT[hsl, hp, isl],
                            start=True,
                            stop=True,
                            tile_position=(hh * D, 0),
                        )
                        pt = pt_pool.tile([P, TC], bf16, tag="pt")
                        if ENG16[jt] == "A":
                            nc.scalar.activation(pt[:], st[:], EXP, scale=0.125)
                        else:
                            nc.vector.tensor_scalar(
                                out=pt[:].bitcast(i16), in0=st[:],
                                scalar1=SCH_A, scalar2=SCH_B,
                                op0=MULT, op1=ADD,
                            )
                        pts.append(pt)
                        # phase A: q-subtiles 0..2 accumulate immediately
                        if jt == 0:
                            for qs in range(3):
                                acc[qs] = acc_ps.tile(
                                    [P, TC], f32, tag="acc", name=f"acc{qs}")
                        for qs in range(3):
                            nc.tensor.matmul(
                                acc[qs][:, : D + 1],
                                lhsT=pt[:, qs * P : (qs + 1) * P],
                                rhs=Vt[:, jt, head, :],
                                start=(jt == 0),
                                stop=(jt == NT - 1),
                            )

                    def rescale(qs, acc_t):
                        rec = rec_pool.tile([P, 1], f32, tag="rec")
                        nc.vector.reciprocal(out=rec[:], in_=acc_t[:, D : D + 1])
                        nc.vector.tensor_scalar_mul(
                            out=aqs[qs][:, hsl], in0=acc_t[:, :D], scalar1=rec[:]
                        )

                    for qs in range(3):
                        rescale(qs, acc[qs])
                    # phase B: q-subtile 3 from retained pt tiles
                    acc[3] = acc_ps.tile([P, TC], f32, tag="acc", name="acc3")
                    for jt in range(NT):
                        nc.tensor.matmul(
                            acc[3][:, : D + 1],
                            lhsT=pts[jt][:, 3 * P : 4 * P],
                            rhs=Vt[:, jt, head, :],
                            start=(jt == 0),
                            stop=(jt == NT - 1),
                        )
                    rescale(3, acc[3])

                # transpose [q, c-pair] -> attnT [c-pair, q] via DMA XBAR
                for qs in range(4):
                    tt = ic * 4 + qs
                    nc.sync.dma_start(
                        out=attnT[:, hp, tt * P : (tt + 1) * P],
                        in_=aqs[qs][:],
                        transpose=True,
                    )
                # overlap helpers: next head-pair's projection slots, or the
                # output projection once the last pair's attnT chunk is done
                if hp < NPAIR - 1:
                    emit_slot(hp + 1, ic, "k")
                    emit_slot(hp + 1, ic, "q")
                else:
                    for qs in range(4):
                        emit_proj(ic * 4 + qs)

    nc.finalize()
    return nc


def _host_inputs(x, rope_cos, rope_sin, W_qkv, b_qkv, W_proj, b_proj, num_special):
    ns = int(num_special)
    cos_pad = np.ones((N, D), np.float32)
    sin_pad = np.zeros((N, D), np.float32)
    cos_pad[ns:] = rope_cos
    sin_pad[ns:] = rope_sin
    cos_tab = np.tile(np.ascontiguousarray(cos_pad.T), (2, 1))
    sin_tab = np.tile(np.ascontiguousarray(sin_pad.T), (2, 1))
    # signed swap: out[d] = -in[d+32] (d%64<32) else +in[d-32], per 64-block
    sg = np.zeros((P, P), np.float32)
    for blk in range(2):
        o = blk * 64
        for d in range(32):
            sg[o + d + 32, o + d] = -1.0      # out[d] = -in[d+32]
            sg[o + d, o + d + 32] = 1.0       # out[d+32] = +in[d]

    in_maps = []
    for core in range(8):
        b, g = core // 2, core % 2
        hs = list(range(HPC * g, HPC * g + HPC))
        cols_qk = []
        for mt in range(6):
            s, hp = (0, mt) if mt < 3 else (1, mt - 3)
            for half in range(2):
                h = hs[2 * hp + half]
                cols_qk.extend(s * 768 + h * 64 + d for d in range(D))
        cols_qk = np.array(cols_qk)
        cols_v = np.array([2 * 768 + hs[i // 64] * 64 + (i % 64) for i in range(384)])
        rows_p = np.array(
            [hs[2 * ko + half] * 64 + d
             for ko in range(3) for half in range(2) for d in range(D)]
        )
        in_maps.append({
            "xT": np.ascontiguousarray(x[b].T),
            "w_qk": np.ascontiguousarray(
                W_qkv[:, cols_qk].reshape(6, P, 768).transpose(1, 0, 2)),
            "w_v": np.ascontiguousarray(
                W_qkv[:, cols_v].reshape(6, P, 384).transpose(1, 0, 2)),
            "w_p": np.ascontiguousarray(
                W_proj[rows_p].reshape(3, P, 768).transpose(1, 0, 2)
            ).astype(ml_dtypes.bfloat16),
            "b_qk_t": np.ascontiguousarray(
                b_qkv[cols_qk].reshape(6, P).T),
            "cos_tab": cos_tab,
            "sin_tab": sin_tab,
            "swap_sign": sg,
        })
    return in_maps


def kernel(x, rope_cos, rope_sin, W_qkv, b_qkv, W_proj, b_proj, num_special):
    global LAST_RESULTS
    from concourse.bass_utils import run_bass_kernel_spmd

    x = np.asarray(x, np.float32)
    W_qkv = np.asarray(W_qkv, np.float32)
    b_qkv = np.asarray(b_qkv, np.float32)
    W_proj = np.asarray(W_proj, np.float32)
    b_proj = np.asarray(b_proj, np.float32)
    if "nc" not in _NC_CACHE:
        _NC_CACHE["nc"] = _build_nc()
    nc = _NC_CACHE["nc"]

    in_maps = _host_inputs(
        x, np.asarray(rope_cos, np.float32), np.asarray(rope_sin, np.float32),
        W_qkv, b_qkv, W_proj, b_proj, num_special,
    )
    trace = bool(int(os.environ.get("KERNEL_TRACE", "0")))
    res = run_bass_kernel_spmd(nc, in_maps, core_ids=list(range(8)), trace=trace)
    LAST_RESULTS = res

    # v-bias folded through the projection: softmax rows sum to 1, so
    # A@(v+b_v) = A@v + b_v; (b_v @ W_proj + b_proj) is a constant row.
    bp_eff = (b_qkv[2 * C :].astype(np.float64) @ W_proj.astype(np.float64)
              + b_proj.astype(np.float64)).astype(np.float32)
    out = np.empty((B, N, C), np.float32)
    for b in range(B):
        out[b] = res.results[2 * b]["y"] + res.results[2 * b + 1]["y"] + bp_eff
    return out
